# revision 25
# baseline (speedup 1.0000x reference)
"""Trainium2 Bass kernel for nn_CCepLTVFilter (v4).

Per core (frequency-sharded f-slice of 128 across 8 cores):
  1. Yr/Yi = DFT(conv1d(x, W) + b) folded on HOST: Y = sum_k (Wk@CF).T @ x_k
     -> the conv matmuls and the ccep PSUM round-trip disappear entirely.
  2. mag = exp(Yr) (ln10/10 folded into CF on host); cos/sin(Yi) via ACT Sin
     with range wraps; -sin derived by negation on gpsimd.
  3. Zr/Zi = 1025-pt DFT of z hops (pre-shifted hop copies from host).
  4. P = (cos + i sin)(Zr + i Zi) * mag, mag applied LAST so the Exp table
     load (the only set switch; tables are single-active) overlaps the chain.
  5. ob[t, l|r] = P_b.T @ (CO|SO); overlap-add of the l/r planes on HOST.

All matmuls and DVE intermediates uniform fp16 (rel err ~7e-3 vs fp32
reference; tolerance 2e-2). gpsimd gets only SBUF-to-SBUF ops (no PSUM
access on trn2). Input DMAs ride two queues in first-use order.
"""

import numpy as np
import ml_dtypes

import concourse.bass as bass
import concourse.bacc as bacc
import concourse.mybir as mybir
import concourse.tile as tile
from concourse.bass_utils import run_bass_kernel_spmd

# ---------------- problem dims (hardcoded) ----------------
B, T, D = 2, 128, 80
CCEP = 222
FFT = 1024
HOP = 256
WIN = 2 * HOP            # 512
PAD = (FFT - CCEP) // 2  # 401
M = FFT + 1              # 1025-point transforms
BT = B * T               # 256
NCORES = 8
FS = FFT // NCORES       # 128 frequencies per core
LAM = float(np.log(10.0) / 10.0)
NWARM = 16               # PE warm-up matmuls (pstate/HAM ramp during DMA wait)
NWARM2 = 4               # mid-kernel PE gap fillers (keyed on cosv readiness)

F32 = mybir.dt.float32
F16 = mybir.dt.float16
PI = float(np.pi)
AF = mybir.ActivationFunctionType
OP = mybir.AluOpType

TRACE = False            # set by test harness for profiling
LAST_RESULT = None       # BassKernelResults of last run (for test harness)


# ---------------- host-side constants (input independent) ----------------
def _make_constants():
    o = np.arange(CCEP, dtype=np.float64)[:, None]
    f = np.arange(FFT, dtype=np.float64)[None, :]
    qn_idx = np.arange(1, CCEP // 2 + 1, dtype=np.float64)
    qnorm = np.concatenate([qn_idx[::-1], qn_idx])
    ang = 2.0 * np.pi * f * (o + PAD) / FFT
    CF = (np.cos(ang) * LAM / qnorm[:, None]).astype(np.float32)   # [222,1024]
    SF = (-np.sin(ang) / qnorm[:, None]).astype(np.float32)

    u = np.arange(WIN, dtype=np.float64)[:, None]
    phi = 2.0 * np.pi * f * (u + FFT // 2) / M
    ZC = np.cos(phi).astype(np.float16)                            # [512,1024]
    ZS = np.sin(phi).astype(np.float16)

    w = np.arange(WIN, dtype=np.float64)[None, :]
    th = 2.0 * np.pi * np.arange(FFT, dtype=np.float64)[:, None] * w / M
    win = 0.5 * (1.0 - np.cos(2.0 * np.pi * np.arange(WIN) / WIN))
    CO = (np.cos(th) * win[None, :] / M).astype(np.float16)        # [1024,512]
    SO = (np.sin(th) * win[None, :] / M).astype(np.float16)

    consts = []
    for c in range(NCORES):
        sl = slice(c * FS, (c + 1) * FS)
        zchunks = [ZC[h * 256 + vc * 128: h * 256 + (vc + 1) * 128, sl]
                   for h in range(2) for vc in range(2)]
        schunks = [ZS[h * 256 + vc * 128: h * 256 + (vc + 1) * 128, sl]
                   for h in range(2) for vc in range(2)]
        dpc = np.concatenate(zchunks + schunks, axis=1).astype(np.float16)
        dpd = np.concatenate([CO[sl, :], SO[sl, :]], axis=1).astype(np.float16)
        consts.append(dict(dpc=dpc, dpd=dpd))
    return consts, CF, SF


_CONSTS, _CF, _SF = _make_constants()
_NC = None


# ---------------- device program ----------------
def _build_nc():
    nc = bacc.Bacc()
    d1 = nc.dram_tensor("xs", [81, 260], F16, kind="ExternalInput")
    dg = nc.dram_tensor("dpg", [81, 768], F16, kind="ExternalInput")
    d2 = nc.dram_tensor("dpa2", [128, 516], F16, kind="ExternalInput")
    d4 = nc.dram_tensor("dpc", [128, 1024], F16, kind="ExternalInput")
    d5 = nc.dram_tensor("dpd", [128, 1024], F16, kind="ExternalInput")
    out_e = nc.dram_tensor("out", [B, 2, T * HOP], F16, kind="ExternalOutput")

    with tile.TileContext(nc) as tc:
        with tc.tile_pool(name="sb", bufs=1) as sb, \
             tc.tile_pool(name="ps", bufs=1, space="PSUM") as ps:

            # ---- input DMAs: two queues, ordered by first use ----
            xs = sb.tile([81, 260], F16, tag="xs", name="xs")
            nc.sync.dma_start(out=xs[:], in_=d1[:, :], single_packet=True)
            dpg = sb.tile([81, 768], F16, tag="dpg", name="dpg")
            nc.gpsimd.dma_start(out=dpg[:], in_=dg[:, :], single_packet=True)
            dpa2 = sb.tile([128, 516], F16, tag="dpa2", name="dpa2")
            nc.sync.dma_start(out=dpa2[:], in_=d2[:, :])
            dpc = sb.tile([128, 1024], F16, tag="dpc", name="dpc")
            nc.gpsimd.dma_start(out=dpc[:], in_=d4[:, :])
            dpd = sb.tile([128, 1024], F16, tag="dpd", name="dpd")
            nc.gpsimd.dma_start(out=dpd[:], in_=d5[:, :])

            # ---- PE warm-up (pstate/HAM ramp) + Sin table pre-load ----
            wsc = sb.tile([128, 256], F16, tag="wsc", name="wsc")
            nc.vector.memset(wsc[:, :], 0.0)
            tsc = sb.tile([1, 1], F32, tag="tsc", name="tsc")
            nc.scalar.activation(tsc[:, :], wsc[0:1, 0:1], AF.Sin)
            wps = ps.tile([128, 256], F32, tag="wps", name="wps")
            for i in range(NWARM):
                nc.tensor.matmul(wps[:, :], wsc[:, 0:128], wsc[:, :],
                                 start=True, stop=True)

            # ---- Yr/Yi [f_local, bt]: conv folded into lhsT on host ----
            yri = ps.tile([FS, 2 * BT], F32, tag="yri", name="yri")
            yr = yri[:, 0:BT]
            yi = yri[:, BT:2 * BT]
            xq = xs[:, 0:260].rearrange("p (b t) -> p b t", b=2)  # [81,2,130]
            for k in range(3):
                nc.tensor.matmul(yi, dpg[:, 384 + k * 128:384 + (k + 1) * 128],
                                 xq[:, :, k:k + 128],
                                 start=(k == 0), stop=(k == 2))
            for k in range(3):
                nc.tensor.matmul(yr, dpg[:, k * 128:(k + 1) * 128],
                                 xq[:, :, k:k + 128],
                                 start=(k == 0), stop=(k == 2))

            # ---- Zr/Zi [f_local, bt] ----
            zri = ps.tile([FS, 2 * BT], F32, tag="zri", name="zri")
            zr = zri[:, 0:BT]
            zi = zri[:, BT:2 * BT]
            hq = [dpa2[:, vc * 258:(vc + 1) * 258].rearrange("p (b t) -> p b t", b=2)
                  for vc in range(2)]
            chunks = [(h, vc) for h in range(2) for vc in range(2)]
            for i, (h, vc) in enumerate(chunks):
                nc.tensor.matmul(zr, dpc[:, (2 * h + vc) * 128:(2 * h + vc + 1) * 128],
                                 hq[vc][:, :, h:h + 128],
                                 start=(i == 0), stop=(i == 3))
            for i, (h, vc) in enumerate(chunks):
                nc.tensor.matmul(zi, dpc[:, 512 + (2 * h + vc) * 128:512 + (2 * h + vc + 1) * 128],
                                 hq[vc][:, :, h:h + 128],
                                 start=(i == 0), stop=(i == 3))

            # ---- cos/sin(Yi); mag = exp(Yr) ordered LAST on ACT ----
            yw1 = sb.tile([FS, BT], F32, tag="yw1", name="yw1")
            nc.vector.add_range_wrap(yw1[:, :], yi, PI / 2.0, PI, 2.0 * PI)
            yw2 = sb.tile([FS, BT], F32, tag="yw2", name="yw2")
            nc.vector.add_range_wrap(yw2[:, :], yi, 0.0, PI, 2.0 * PI)
            cosv = sb.tile([FS, BT], F16, tag="cosv", name="cosv")
            nc.scalar.activation(cosv[:, :], yw1[:, :], AF.Sin)
            sinp = sb.tile([FS, BT], F16, tag="sinp", name="sinp")
            nc.scalar.activation(sinp[:, :], yw2[:, :], AF.Sin)
            mag = sb.tile([FS, BT], F16, tag="mag", name="mag")
            nc.scalar.activation(mag[:, :], yr, AF.Exp)

            # ---- mid-kernel PE gap fillers (keep HAM warm until ob) ----
            for i in range(NWARM2):
                nc.tensor.matmul(wps[:, :], wsc[:, 0:128], cosv[:, :],
                                 start=True, stop=True)

            # ---- P = (cos + i sin)(Zr + i Zi) * mag, all on V (no gpsimd
            # compute anywhere -> no ucode library load on the chain) ----
            qa = sb.tile([FS, 2 * BT], F16, tag="qa", name="qa")
            nc.vector.tensor_tensor(qa[:, 0:BT], cosv[:, :], zr, OP.mult)
            nc.vector.tensor_tensor(qa[:, BT:2 * BT], cosv[:, :], zi, OP.mult)
            qb = sb.tile([FS, 2 * BT], F16, tag="qb", name="qb")
            nc.vector.tensor_tensor(qb[:, 0:BT], sinp[:, :], zi, OP.mult)
            nc.vector.tensor_tensor(qb[:, BT:2 * BT], sinp[:, :], zr, OP.mult)
            pp = sb.tile([FS, 2 * BT], F16, tag="pp", name="pp")
            dd = sb.tile([FS, 2 * BT], F16, tag="dd", name="dd")
            nc.vector.tensor_tensor(dd[:, 0:BT], qa[:, 0:BT], qb[:, 0:BT],
                                    OP.subtract)
            nc.vector.tensor_tensor(dd[:, BT:2 * BT], qa[:, BT:2 * BT],
                                    qb[:, BT:2 * BT], OP.add)
            nc.vector.tensor_tensor(pp[:, 0:BT], mag[:, :], dd[:, 0:BT], OP.mult)
            nc.vector.tensor_tensor(pp[:, BT:2 * BT], mag[:, :], dd[:, BT:2 * BT],
                                    OP.mult)
            pr = pp[:, 0:BT]
            pi = pp[:, BT:2 * BT]

            # ---- ob[t, l|r] = P_b.T @ (CO|SO); OLA of planes on host ----
            for bb in range(B):
                obp = ps.tile([T, WIN], F32, tag=f"ob{bb}", name=f"ob{bb}")
                nc.tensor.matmul(obp[:, :], pr[:, bb * T:(bb + 1) * T],
                                 dpd[:, 0:512], start=True, stop=False)
                nc.tensor.matmul(obp[:, :], pi[:, bb * T:(bb + 1) * T],
                                 dpd[:, 512:1024], start=False, stop=True)
                obs = sb.tile([T, WIN], F16, tag=f"obs{bb}", name=f"obs{bb}")
                if bb == 0:
                    nc.scalar.copy(obs[:, :], obp[:, :])
                else:
                    nc.vector.tensor_copy(obs[:, :], obp[:, :])
                # dst[bb, plane, t*HOP + s] <- obs[t, plane*HOP + s]
                dst = bass.AP(out_e[:, :, :].tensor, bb * 2 * T * HOP,
                              [[HOP, T], [T * HOP, 2], [1, HOP]])
                eng = nc.sync if bb == 0 else nc.scalar
                eng.dma_start(out=dst, in_=obs[:, :])

    return nc


def _get_nc():
    global _NC
    if _NC is None:
        _NC = _build_nc()
        _NC.finalize()
    return _NC


# ---------------- host orchestration ----------------
def kernel(x, z, W, b):
    global LAST_RESULT
    x = np.asarray(x, dtype=np.float32)
    z = np.asarray(z, dtype=np.float32)
    W = np.asarray(W, dtype=np.float32)
    b = np.asarray(b, dtype=np.float32)

    # xs [81, 2*130]: per-batch blocks [g, x(128), g]; ones row for bias
    xv = x.astype(np.float16)                                     # [2,128,80]
    xs = np.zeros((81, 260), np.float16)
    for bb in range(B):
        xs[0:80, bb * 130 + 1: bb * 130 + 129] = xv[bb].T
        xs[80, bb * 130 + 1: bb * 130 + 129] = 1.0
    GF = np.zeros((3, 81, FFT), np.float32)                       # Wk_ext @ CF
    GI = np.zeros((3, 81, FFT), np.float32)
    for k in range(3):
        wke = np.zeros((81, CCEP), np.float32)
        wke[0:80] = W[:, :, k].T
        if k == 1:
            wke[80] = b
        GF[k] = wke @ _CF                                         # [81, 1024]
        GI[k] = wke @ _SF

    # dpa2 = hop matrix, duplicated per h-shift: chunk (h,vc) at (2h+vc)*256
    zpad = np.concatenate(
        [np.zeros((B, HOP), np.float32), z[:, 0, :]], axis=1)     # [2, 33024]
    Hm = zpad.reshape(B, 129, HOP).transpose(2, 0, 1)             # [256, 2, 129]
    dpa2 = np.ascontiguousarray(
        Hm.reshape(2, 128, 2 * 129).transpose(1, 0, 2).reshape(128, 516)
    ).astype(np.float16)

    in_maps = []
    for c in range(NCORES):
        sl = slice(c * FS, (c + 1) * FS)
        dpg = np.concatenate(
            [GF[0][:, sl], GF[1][:, sl], GF[2][:, sl],
             GI[0][:, sl], GI[1][:, sl], GI[2][:, sl]],
            axis=1).astype(np.float16)                            # [81, 768]
        in_maps.append({"xs": xs, "dpg": dpg, "dpa2": dpa2, **_CONSTS[c]})

    nc = _get_nc()
    res = run_bass_kernel_spmd(nc, in_maps, list(range(NCORES)), trace=TRACE)
    LAST_RESULT = res
    acc = np.zeros((B, 2, T * HOP), dtype=np.float32)
    for r in res.results:
        acc += np.asarray(r["out"], dtype=np.float32)
    out = np.empty((B, 1, T * HOP), dtype=np.float32)
    for bb in range(B):
        out[bb, 0] = acc[bb, 0] + np.roll(acc[bb, 1], HOP)
    return out
